# revision 1
# baseline (speedup 1.0000x reference)
"""Trainium2 Bass kernel for nn_CrossAttention (gnn_message_passing).

Per batch b (B=8, one per NeuronCore), K=16 neighbors, C=64 channels,
M=8192 points:
  query/key/value projections, two small xyz self-attentions (pem/peb),
  relation MLP, softmax over neighbors, weighted sum, residual projection.

Exact restructuring (validated vs reference at ~4e-7 rel err in numpy):
  - ones-row trick: x' = [xyz; 1] folds all biases into matmuls
  - xyz self-attention scores via the bilinear fold
        S[k,j] = x'_k^T A' x'_j,   A' = Wq'^T Wk' / sqrt(C)
  - softmax denominator = ones-component of Y = sum_j exp(S[k,j]) x'_j
  - relu(Z)/den == relu(Z/den) for den>0: division deferred
  - query folded into the key matmul as a PSUM-accumulated correction

Layouts: rows-on-partitions / points-on-free for all PE channel matmuls;
the per-point KxK attention runs in points-on-partitions layout (128/tile)
on the vector engine using free-dim step-0 broadcast access patterns.
"""
import sys
sys.path.insert(0, '/opt/trn_rl_repo')

import numpy as np

B, C, K, M = 8, 64, 16, 8192
T = 512            # points per m-tile
NSUB = T // 128

_CACHE = {}


def _derived_weights(inp):
    """Host-side folding of the tiny channel weights into PE lhsT tensors."""
    f = np.float32

    def wp(Wname, bname):
        return np.concatenate(
            [np.asarray(inp[Wname], f), np.asarray(inp[bname], f)[:, None]], axis=1)

    qWp = wp('q_W', 'q_b')                      # [C,4]
    kW = np.asarray(inp['k_W'], f)
    vW = np.asarray(inp['v_W'], f)
    kb = np.asarray(inp['k_b'], f)
    vb = np.asarray(inp['v_b'], f)
    scale = f(1.0) / np.sqrt(f(C))
    A, Vp = {}, {}
    for tag in ('pm', 'pb'):
        qq = wp(f'{tag}_q_W', f'{tag}_q_b')
        kk2 = wp(f'{tag}_k_W', f'{tag}_k_b')
        A[tag] = ((qq.T @ kk2) * scale).astype(f)
        Vp[tag] = wp(f'{tag}_v_W', f'{tag}_v_b')

    W = {}
    # G production: lhsT [64,128]; X' row (k*4+d) -> G row (attn*64+k*4+d')
    wg = np.zeros((64, 128), f)
    for k in range(K):
        wg[k * 4:k * 4 + 4, k * 4:k * 4 + 4] = A['pm']
        wg[k * 4:k * 4 + 4, 64 + k * 4:64 + k * 4 + 4] = A['pb']
    W['wxg'] = np.concatenate([np.eye(64, dtype=f), wg], axis=1)  # [64,192]

    # key / value blockdiag for a k-pair F block [128,128]
    wk2 = np.zeros((128, 128), f)
    wv2 = np.zeros((128, 128), f)
    for kk in range(2):
        s = kk * 64
        wk2[s:s + 64, s:s + 64] = kW.T
        wv2[s:s + 64, s:s + 64] = vW.T
    W['wk'] = wk2
    W['wv'] = wv2

    # query subtraction (+ k_b): rhs = full X' [64, T]; variant per k-pair b2
    wq = np.zeros((64, K // 2, 128), f)
    for b2 in range(K // 2):
        for kk in range(2):
            r = b2 * 8 + kk * 4
            cs = slice(kk * 64, kk * 64 + 64)
            wq[r:r + 4, b2, cs] = -qWp.T
            wq[r + 3, b2, cs] += kb
    W['wq'] = wq.reshape(64, (K // 2) * 128)

    if np.any(vb != 0):
        wvb = np.zeros((64, K // 2, 128), f)
        for b2 in range(K // 2):
            for kk in range(2):
                wvb[b2 * 8 + kk * 4 + 3, b2, kk * 64:kk * 64 + 64] = vb
        W['wvb'] = wvb.reshape(64, (K // 2) * 128)

    # pem/peb projections: rhs = full Yrows [128, T]; variant per k-pair
    for ia, tag in enumerate(('pm', 'pb')):
        wpe = np.zeros((128, K // 2, 128), f)
        for b2 in range(K // 2):
            for kk in range(2):
                r = ia * 64 + b2 * 8 + kk * 4
                wpe[r:r + 4, b2, kk * 64:kk * 64 + 64] = Vp[tag].T
        W['wpe_' + tag] = wpe.reshape(128, (K // 2) * 128)

    # weight-encoding MLP blockdiag [128,128] + bias vectors [128,1]
    we1 = np.zeros((128, 128), f)
    we2 = np.zeros((128, 128), f)
    for kk in range(2):
        s = kk * 64
        we1[s:s + 64, s:s + 64] = np.asarray(inp['we_W1'], f).T
        we2[s:s + 64, s:s + 64] = np.asarray(inp['we_W2'], f).T
    W['we1'] = we1
    W['we2'] = we2
    W['b1'] = np.tile(np.asarray(inp['we_b1'], f), 2)[:, None]
    W['b2'] = np.tile(np.asarray(inp['we_b2'], f), 2)[:, None]

    W['wones'] = np.vstack([np.eye(64, dtype=f), np.eye(64, dtype=f)])

    reW = np.asarray(inp['re_W'], f)
    reb = np.asarray(inp['re_b'], f)
    if np.any(reb != 0):
        W['wre'] = np.vstack([reW.T, reb[None, :]])   # [65, 64]
        W['_has_reb'] = np.ones(1, f)
    else:
        W['wre'] = np.ascontiguousarray(reW.T)        # [64, 64]
    W['ident'] = np.eye(128, dtype=f)
    return W


def _host_tensors(inp):
    f = np.float32
    xyz = np.asarray(inp['grouped_xyz'], f)        # [B,3,K,M]
    feat = np.asarray(inp['grouped_feature'], f)   # [B,C,K,M]
    Bl, _, Kl, Ml = xyz.shape
    xp = np.empty((Bl, 64, Ml), f)
    xp4 = xp.reshape(Bl, Kl, 4, Ml)
    xp4[:, :, 0:3, :] = xyz.transpose(0, 2, 1, 3)
    xp4[:, :, 3, :] = 1.0
    fr = np.ascontiguousarray(
        feat.reshape(Bl, C, Kl // 2, 2, Ml).transpose(0, 3, 1, 2, 4)
    ).reshape(Bl, 128, Kl // 2, Ml)
    return xp, fr


def build_kernel(Mloc, has_vb, has_reb, wshapes):
    """Emit the Bass program for one core processing Mloc points."""
    import concourse.bacc as bacc
    import concourse.tile as tile
    import concourse.bass as bass
    from concourse import mybir

    f32 = mybir.dt.float32
    AL = mybir.AluOpType
    AF = mybir.ActivationFunctionType
    NT = Mloc // T
    FEATP = 65 if has_reb else 64
    NB = K // 2

    def rap(sl, free_ap):
        """Re-dim a (sliced) AP: keep tensor/offset/partition pair, replace
        free dims (steps in elements)."""
        return bass.AP(tensor=sl.tensor, offset=sl.offset,
                       ap=[list(sl.ap[0])] + [list(p) for p in free_ap])

    nc = bacc.Bacc()
    xp_d = nc.declare_dram_parameter("xp", [64, Mloc], f32, isOutput=False)
    fr_d = nc.declare_dram_parameter("fr", [128, NB, Mloc], f32, isOutput=False)
    wdecl = {}
    for name, shp in wshapes.items():
        wdecl[name] = nc.declare_dram_parameter("w_" + name, list(shp), f32,
                                                isOutput=False)
    out_d = nc.declare_dram_parameter("out", [Mloc, 64], f32, isOutput=True)

    with tile.TileContext(nc) as tc:
        with (
            tc.tile_pool(name="wpool", bufs=1) as wpool,
            tc.tile_pool(name="xf", bufs=2) as xf,
            tc.tile_pool(name="mid", bufs=2) as mid,
            tc.tile_pool(name="attn", bufs=3) as attn,
            tc.tile_pool(name="blk", bufs=3) as blk,
            tc.tile_pool(name="ps_g", bufs=1, space="PSUM") as ps_g,
            tc.tile_pool(name="ps_tr", bufs=1, space="PSUM") as ps_tr,
            tc.tile_pool(name="ps_rv", bufs=1, space="PSUM") as ps_rv,
            tc.tile_pool(name="ps_w", bufs=1, space="PSUM") as ps_w,
            tc.tile_pool(name="ps_acc", bufs=1, space="PSUM") as ps_acc,
        ):
            wsb = {}
            for name, d in wdecl.items():
                t = wpool.tile(list(wshapes[name]), f32, tag="w_" + name)
                nc.gpsimd.dma_start(out=t[:], in_=d[:])
                wsb[name] = t

            for it in range(NT):
                ms = it * T
                xpt = xf.tile([64, T], f32, tag="xpt")
                nc.sync.dma_start(out=xpt[:], in_=xp_d[:, ms:ms + T])
                frt = xf.tile([128, NB, T], f32, tag="frt")
                nc.sync.dma_start(out=frt[:], in_=fr_d[:, :, ms:ms + T])

                # ---- per-128pt attention (m on partitions) ----
                # XT/GT via one data-stationary matmul: out[m, col] =
                # sum_p X'[p, m] * [I64 | wg][p, col]
                yrows = mid.tile([128, T], f32, tag="yrows")
                for s in range(NSUB):
                    c0 = s * 128
                    pxg = ps_tr.tile([128, 192], f32, tag="ptr")
                    nc.tensor.matmul(pxg[:], xpt[:, c0:c0 + 128], wsb['wxg'][:],
                                     start=True, stop=True)
                    xgt = attn.tile([128, 192], f32, tag="xgt")
                    nc.vector.tensor_copy(xgt[:], pxg[:])
                    xt = xgt[:, 0:64]
                    gt = xgt[:, 64:192]

                    yn2 = attn.tile([128, 128], f32, tag="yn2")
                    for ia in range(2):
                        ao = ia * 64
                        mul_eng = nc.vector
                        # SW[m,(k,j,d)] = G[m,k*4+d] * X[m,j*4+d]
                        sw = attn.tile([128, 1024], f32, tag="sw")
                        mul_eng.tensor_tensor(
                            out=rap(sw[:], [[64, 16], [4, 16], [1, 4]]),
                            in0=rap(gt[:, ao:ao + 64],
                                    [[4, 16], [0, 16], [1, 4]]),
                            in1=rap(xt[:], [[0, 16], [4, 16], [1, 4]]),
                            op=AL.mult)
                        # S = sum_d SW
                        ss = attn.tile([128, 256], f32, tag="ss")
                        nc.vector.tensor_reduce(
                            out=ss[:],
                            in_=rap(sw[:], [[4, 256], [1, 4]]),
                            axis=mybir.AxisListType.X, op=AL.add)
                        ee = attn.tile([128, 256], f32, tag="ee")
                        nc.scalar.activation(out=ee[:], in_=ss[:], func=AF.Exp)
                        # YW[m,(k,d,j)] = E[m,k*16+j] * X[m,j*4+d]
                        yw = attn.tile([128, 1024], f32, tag="yw")
                        mul_eng.tensor_tensor(
                            out=rap(yw[:], [[64, 16], [16, 4], [1, 16]]),
                            in0=rap(ee[:], [[16, 16], [0, 4], [1, 16]]),
                            in1=rap(xt[:], [[0, 16], [1, 4], [4, 16]]),
                            op=AL.mult)
                        yu = attn.tile([128, 64], f32, tag="yu")
                        nc.vector.tensor_reduce(
                            out=yu[:],
                            in_=rap(yw[:], [[16, 64], [1, 16]]),
                            axis=mybir.AxisListType.X, op=AL.add)
                        rec = attn.tile([128, 16], f32, tag="rec")
                        nc.vector.reciprocal(
                            out=rec[:],
                            in_=rap(yu[:, 3:4], [[4, 16]]))
                        yns = yn2[:, ao:ao + 64]
                        mul_eng.tensor_tensor(
                            out=rap(yns, [[4, 16], [1, 4]]),
                            in0=rap(yu[:], [[4, 16], [1, 4]]),
                            in1=rap(rec[:], [[1, 16], [0, 4]]),
                            op=AL.mult)
                    pyn = ps_tr.tile([128, 128], f32, tag="pyn0")
                    nc.tensor.transpose(pyn[:], yn2[:], wsb['ident'][:])
                    nc.vector.tensor_copy(yrows[:, c0:c0 + 128], pyn[:])

                # ---- main pipeline per k-pair block ----
                pnum = ps_acc.tile([128, T], f32, tag="pnum")
                for b2 in range(NB):
                    w128 = slice(b2 * 128, (b2 + 1) * 128)
                    pr = ps_rv.tile([128, T], f32, tag="pr")
                    nc.tensor.matmul(pr[:], wsb['wk'][:], frt[:, b2, :],
                                     start=True, stop=False)
                    nc.tensor.matmul(pr[:], wsb['wq'][:, w128], xpt[:],
                                     start=False, stop=True)
                    pv = ps_rv.tile([128, T], f32, tag="pv")
                    if has_vb:
                        nc.tensor.matmul(pv[:], wsb['wv'][:], frt[:, b2, :],
                                         start=True, stop=False)
                        nc.tensor.matmul(pv[:], wsb['wvb'][:, w128], xpt[:],
                                         start=False, stop=True)
                    else:
                        nc.tensor.matmul(pv[:], wsb['wv'][:], frt[:, b2, :],
                                         start=True, stop=True)

                    ppe = ps_w.tile([128, T], f32, tag="ppe")
                    nc.tensor.matmul(ppe[:], wsb['wpe_pm'][:, w128], yrows[:],
                                     start=True, stop=True)
                    pem = blk.tile([128, T], f32, tag="pem")
                    nc.scalar.activation(out=pem[:], in_=ppe[:], func=AF.Relu)
                    ppb = ps_w.tile([128, T], f32, tag="ppe")
                    nc.tensor.matmul(ppb[:], wsb['wpe_pb'][:, w128], yrows[:],
                                     start=True, stop=True)
                    peb = blk.tile([128, T], f32, tag="peb")
                    nc.scalar.activation(out=peb[:], in_=ppb[:], func=AF.Relu)

                    dd = blk.tile([128, T], f32, tag="dd")
                    nc.vector.tensor_tensor(out=dd[:], in0=pr[:], in1=pem[:],
                                            op=AL.mult)
                    rr = blk.tile([128, T], f32, tag="rr")
                    nc.vector.tensor_tensor(out=rr[:], in0=dd[:], in1=peb[:],
                                            op=AL.add)
                    vv = blk.tile([128, T], f32, tag="vv")
                    nc.vector.tensor_tensor(out=vv[:], in0=pv[:], in1=peb[:],
                                            op=AL.add)

                    pw1 = ps_w.tile([128, T], f32, tag="pw1")
                    nc.tensor.matmul(pw1[:], wsb['we1'][:], rr[:],
                                     start=True, stop=True)
                    r1 = blk.tile([128, T], f32, tag="r1")
                    nc.scalar.activation(out=r1[:], in_=pw1[:], func=AF.Relu,
                                         bias=wsb['b1'][:], scale=1.0)
                    pw2 = ps_w.tile([128, T], f32, tag="pw1")
                    nc.tensor.matmul(pw2[:], wsb['we2'][:], r1[:],
                                     start=True, stop=True)
                    ew = blk.tile([128, T], f32, tag="ew")
                    nc.scalar.activation(out=ew[:], in_=pw2[:], func=AF.Exp,
                                         bias=wsb['b2'][:], scale=1.0)

                    nm = blk.tile([128, T], f32, tag="nm")
                    nc.vector.tensor_tensor(out=nm[:], in0=ew[:], in1=vv[:],
                                            op=AL.mult)
                    nc.tensor.matmul(pnum[0:64, :], wsb['wones'][:], nm[:],
                                     start=(b2 == 0), stop=(b2 == NB - 1),
                                     skip_group_check=True)
                    nc.tensor.matmul(pnum[64:128, :], wsb['wones'][:], ew[:],
                                     start=(b2 == 0), stop=(b2 == NB - 1),
                                     skip_group_check=True)

                # ---- feature = relu(num/den); final projection ----
                rden = mid.tile([64, T], f32, tag="rden")
                nc.vector.reciprocal(out=rden[:], in_=pnum[64:128, :])
                ff = mid.tile([FEATP, T], f32, tag="ff")
                nc.vector.scalar_tensor_tensor(
                    out=ff[0:64, :], in0=pnum[0:64, :], scalar=0.0,
                    in1=rden[:], op0=AL.max, op1=AL.mult)
                if has_reb:
                    nc.vector.memset(ff[64:65, :], 1.0)

                pout = ps_acc.tile([128, NSUB * 64], f32, tag="pout")
                for s in range(NSUB):
                    nc.tensor.matmul(pout[:, s * 64:(s + 1) * 64],
                                     ff[:, s * 128:(s + 1) * 128], wsb['wre'][:],
                                     start=True, stop=True)
                osb = mid.tile([128, NSUB * 64], f32, tag="osb")
                nc.vector.tensor_copy(osb[:], pout[:])
                nc.sync.dma_start(
                    out=bass.AP(tensor=out_d[:].tensor, offset=ms * 64,
                            ap=[[64, 128], [128 * 64, NSUB], [1, 64]]),
                    in_=rap(osb[:], [[64, NSUB], [1, 64]]))

    nc.finalize()
    return nc


LAST_RESULT = None


def kernel(**inputs):
    import os
    from concourse.bass_utils import run_bass_kernel_spmd

    W = _derived_weights(inputs)
    xp, fr = _host_tensors(inputs)
    Bl = xp.shape[0]
    Mloc = xp.shape[2]
    has_vb = 'wvb' in W
    has_reb = '_has_reb' in W
    wshapes = {k: v.shape for k, v in W.items() if not k.startswith('_')}

    key = (Mloc, has_vb, has_reb)
    if key not in _CACHE:
        _CACHE[key] = build_kernel(Mloc, has_vb, has_reb, wshapes)
    nc = _CACHE[key]

    in_maps = []
    for b in range(Bl):
        m = {"xp": xp[b], "fr": fr[b]}
        for name in wshapes:
            m["w_" + name] = W[name]
        in_maps.append(m)

    res = run_bass_kernel_spmd(nc, in_maps, core_ids=list(range(Bl)),
                               trace=bool(os.environ.get("KERNEL_TRACE")))
    global LAST_RESULT
    LAST_RESULT = res
    out = np.stack([res.results[b]["out"] for b in range(Bl)], axis=0)
    return out.astype(np.float32)



# revision 19
# speedup vs baseline: 1.3313x; 1.3313x over previous
"""Trainium2 Bass kernel for nn_CrossAttention (gnn_message_passing).

Per batch b (B=8, one per NeuronCore), K=16 neighbors, C=64 channels,
M=8192 points:
  query/key/value projections, two small xyz self-attentions (pem/peb),
  relation MLP, softmax over neighbors, weighted sum, residual projection.

Exact restructuring (validated vs reference at ~4e-7 rel err in numpy):
  - ones-row trick: x' = [xyz; 1] folds all biases into matmuls
  - xyz self-attention scores via the bilinear fold
        S[k,j] = x'_k^T A' x'_j,   A' = Wq'^T Wk' / sqrt(C)
  - softmax denominator = ones-component of Y = sum_j exp(S[k,j]) x'_j
  - relu(Z)/den == relu(Z/den) for den>0: division deferred
  - query folded into the key matmul as a PSUM-accumulated correction

Layouts: rows-on-partitions / points-on-free for all PE channel matmuls;
the per-point KxK attention runs in points-on-partitions layout (128/tile)
on the vector engine using free-dim step-0 broadcast access patterns.
"""
import sys
sys.path.insert(0, '/opt/trn_rl_repo')

import numpy as np

B, C, K, M = 8, 64, 16, 8192
T = 512            # points per m-tile
NSUB = T // 128

_CACHE = {}


def _derived_weights(inp):
    """Host-side folding of the tiny channel weights into PE lhsT tensors."""
    f = np.float32

    def wp(Wname, bname):
        return np.concatenate(
            [np.asarray(inp[Wname], f), np.asarray(inp[bname], f)[:, None]], axis=1)

    qWp = wp('q_W', 'q_b')                      # [C,4]
    kW = np.asarray(inp['k_W'], f)
    vW = np.asarray(inp['v_W'], f)
    kb = np.asarray(inp['k_b'], f)
    vb = np.asarray(inp['v_b'], f)
    scale = f(1.0) / np.sqrt(f(C))
    A, Vp = {}, {}
    for tag in ('pm', 'pb'):
        qq = wp(f'{tag}_q_W', f'{tag}_q_b')
        kk2 = wp(f'{tag}_k_W', f'{tag}_k_b')
        A[tag] = ((qq.T @ kk2) * scale).astype(f)
        Vp[tag] = wp(f'{tag}_v_W', f'{tag}_v_b')

    W = {}
    # G production: lhsT [64,128]; X' row (k*4+d) -> G row (attn*64+k*4+d')
    wg = np.zeros((64, 128), f)
    for k in range(K):
        wg[k * 4:k * 4 + 4, k * 4:k * 4 + 4] = A['pm']
        wg[k * 4:k * 4 + 4, 64 + k * 4:64 + k * 4 + 4] = A['pb']
    W['wxg'] = np.concatenate([np.eye(64, dtype=f), wg], axis=1)  # [64,192]

    # key / value blockdiag for a k-pair F block [128,128]
    wk2 = np.zeros((128, 128), f)
    wv2 = np.zeros((128, 128), f)
    for kk in range(2):
        s = kk * 64
        wk2[s:s + 64, s:s + 64] = kW.T
        wv2[s:s + 64, s:s + 64] = vW.T
    W['wk'] = wk2
    W['wv'] = wv2

    # query subtraction (+ k_b): rhs = full X' [64, T]; variant per k-pair b2
    wq = np.zeros((64, K // 2, 128), f)
    for b2 in range(K // 2):
        for kk in range(2):
            r = b2 * 8 + kk * 4
            cs = slice(kk * 64, kk * 64 + 64)
            wq[r:r + 4, b2, cs] = -qWp.T
            wq[r + 3, b2, cs] += kb
    W['wq'] = wq.reshape(64, (K // 2) * 128)

    if np.any(vb != 0):
        wvb = np.zeros((64, K // 2, 128), f)
        for b2 in range(K // 2):
            for kk in range(2):
                wvb[b2 * 8 + kk * 4 + 3, b2, kk * 64:kk * 64 + 64] = vb
        W['wvb'] = wvb.reshape(64, (K // 2) * 128)

    # pem/peb projections: rhs = full Yrows [128, T]; variant per k-pair
    for ia, tag in enumerate(('pm', 'pb')):
        wpe = np.zeros((128, K // 2, 128), f)
        for b2 in range(K // 2):
            for kk in range(2):
                r = ia * 64 + b2 * 8 + kk * 4
                wpe[r:r + 4, b2, kk * 64:kk * 64 + 64] = Vp[tag].T
        W['wpe_' + tag] = wpe.reshape(128, (K // 2) * 128)

    # weight-encoding MLP blockdiag [128,128] + bias vectors [128,1]
    we1 = np.zeros((128, 128), f)
    we2 = np.zeros((128, 128), f)
    for kk in range(2):
        s = kk * 64
        we1[s:s + 64, s:s + 64] = np.asarray(inp['we_W1'], f).T
        we2[s:s + 64, s:s + 64] = np.asarray(inp['we_W2'], f).T
    W['we1'] = we1
    W['we2'] = we2
    W['b1'] = np.tile(np.asarray(inp['we_b1'], f), 2)[:, None]
    W['b2'] = np.tile(np.asarray(inp['we_b2'], f), 2)[:, None]

    W['wones'] = np.vstack([np.eye(64, dtype=f), np.eye(64, dtype=f)])

    reW = np.asarray(inp['re_W'], f)
    reb = np.asarray(inp['re_b'], f)
    if np.any(reb != 0):
        W['wre'] = np.vstack([reW.T, reb[None, :]])   # [65, 64]
        W['_has_reb'] = np.ones(1, f)
    else:
        W['wre'] = np.ascontiguousarray(reW.T)        # [64, 64]
    W['ident'] = np.eye(128, dtype=f)
    return W


def _host_tensors(inp):
    f = np.float32
    xyz = np.asarray(inp['grouped_xyz'], f)        # [B,3,K,M]
    feat = np.asarray(inp['grouped_feature'], f)   # [B,C,K,M]
    Bl, _, Kl, Ml = xyz.shape
    xp = np.empty((Bl, 64, Ml), f)
    xp4 = xp.reshape(Bl, Kl, 4, Ml)
    xp4[:, :, 0:3, :] = xyz.transpose(0, 2, 1, 3)
    xp4[:, :, 3, :] = 1.0
    fr = np.ascontiguousarray(
        feat.reshape(Bl, C, Kl // 2, 2, Ml).transpose(0, 3, 1, 2, 4)
    ).reshape(Bl, 128, Kl // 2, Ml)
    import ml_dtypes
    return xp.astype(ml_dtypes.bfloat16), fr.astype(ml_dtypes.bfloat16)


def build_kernel(Mloc, has_vb, has_reb, wshapes):
    """Emit the Bass program for one core processing Mloc points."""
    import concourse.bacc as bacc
    import concourse.tile as tile
    import concourse.bass as bass
    from concourse import mybir

    f32 = mybir.dt.float32
    bf = mybir.dt.bfloat16
    AL = mybir.AluOpType
    AF = mybir.ActivationFunctionType
    NT = Mloc // T
    FEATP = 65 if has_reb else 64
    NB = K // 2

    def rap(sl, free_ap):
        """Re-dim a (sliced) AP: keep tensor/offset/partition pair, replace
        free dims (steps in elements)."""
        return bass.AP(tensor=sl.tensor, offset=sl.offset,
                       ap=[list(sl.ap[0])] + [list(p) for p in free_ap])

    nc = bacc.Bacc()
    xp_d = nc.declare_dram_parameter("xp", [64, Mloc], bf, isOutput=False)
    fr_d = nc.declare_dram_parameter("fr", [128, NB, Mloc], bf, isOutput=False)
    wdecl = {}
    for name, shp in wshapes.items():
        wdecl[name] = nc.declare_dram_parameter("w_" + name, list(shp), bf,
                                                isOutput=False)
    out_d = nc.declare_dram_parameter("out", [Mloc, 64], f32, isOutput=True)

    with tile.TileContext(nc) as tc, nc.allow_low_precision(reason='bf16'):
        with (
            tc.tile_pool(name="wpool", bufs=1) as wpool,
            tc.tile_pool(name="xf", bufs=2) as xf,
            tc.tile_pool(name="mid", bufs=2) as mid,
            tc.tile_pool(name="attn", bufs=3) as attn,
            tc.tile_pool(name="blk", bufs=3) as blk,
            tc.tile_pool(name="ps_g", bufs=1, space="PSUM") as ps_g,
            tc.tile_pool(name="ps_tr", bufs=1, space="PSUM") as ps_tr,
            tc.tile_pool(name="ps_rv", bufs=1, space="PSUM") as ps_rv,
            tc.tile_pool(name="ps_w", bufs=1, space="PSUM") as ps_w,
            tc.tile_pool(name="ps_acc", bufs=1, space="PSUM") as ps_acc,
        ):
            wsb = {}
            for name, d in wdecl.items():
                t = wpool.tile(list(wshapes[name]), bf, tag="w_" + name)
                nc.gpsimd.dma_start(out=t[:], in_=d[:])
                wsb[name] = t

            for it in range(NT):
                ms = it * T
                xpt = xf.tile([64, T], bf, tag="xpt")
                nc.sync.dma_start(out=xpt[:], in_=xp_d[:, ms:ms + T])
                frt = xf.tile([128, NB, T], bf, tag="frt")
                nc.sync.dma_start(out=frt[:], in_=fr_d[:, :, ms:ms + T])

                # ---- per-128pt attention (m on partitions) ----
                # XT/GT via one data-stationary matmul: out[m, col] =
                # sum_p X'[p, m] * [I64 | wg][p, col]
                yrows = mid.tile([128, T], bf, tag="yrows")
                for s in range(NSUB):
                    c0 = s * 128
                    pxg = ps_tr.tile([128, 192], f32, tag="ptr")
                    nc.tensor.matmul(pxg[:], xpt[:, c0:c0 + 128], wsb['wxg'][:],
                                     start=True, stop=True)
                    xgt = attn.tile([128, 192], bf, tag="xgt")
                    nc.vector.tensor_copy(xgt[:], pxg[:])
                    xt = xgt[:, 0:64]
                    gt = xgt[:, 64:192]

                    yn2 = attn.tile([128, 128], bf, tag="yn2")
                    for ia in range(2):
                        ao = ia * 64
                        mul_eng = nc.vector
                        # SW[m,(k,j,d)] = G[m,k*4+d] * X[m,j*4+d]
                        sw = attn.tile([128, 1024], bf, tag="sw")
                        mul_eng.tensor_tensor(
                            out=rap(sw[:], [[64, 16], [4, 16], [1, 4]]),
                            in0=rap(gt[:, ao:ao + 64],
                                    [[4, 16], [0, 16], [1, 4]]),
                            in1=rap(xt[:], [[0, 16], [4, 16], [1, 4]]),
                            op=AL.mult)
                        # S = sum_d SW
                        ss = attn.tile([128, 256], bf, tag="ss")
                        nc.vector.tensor_reduce(
                            out=ss[:],
                            in_=rap(sw[:], [[4, 256], [1, 4]]),
                            axis=mybir.AxisListType.X, op=AL.add)
                        ee = attn.tile([128, 256], bf, tag="ee")
                        nc.scalar.activation(out=ee[:], in_=ss[:], func=AF.Exp)
                        # YW[m,(k,d,j)] = E[m,k*16+j] * X[m,j*4+d]
                        yw = attn.tile([128, 1024], bf, tag="yw")
                        mul_eng.tensor_tensor(
                            out=rap(yw[:], [[64, 16], [16, 4], [1, 16]]),
                            in0=rap(ee[:], [[16, 16], [0, 4], [1, 16]]),
                            in1=rap(xt[:], [[0, 16], [1, 4], [4, 16]]),
                            op=AL.mult)
                        yu = attn.tile([128, 64], bf, tag="yu")
                        nc.vector.tensor_reduce(
                            out=yu[:],
                            in_=rap(yw[:], [[16, 64], [1, 16]]),
                            axis=mybir.AxisListType.X, op=AL.add)
                        rec = attn.tile([128, 16], bf, tag="rec")
                        nc.vector.reciprocal(
                            out=rec[:],
                            in_=rap(yu[:, 3:4], [[4, 16]]))
                        yns = yn2[:, ao:ao + 64]
                        mul_eng.tensor_tensor(
                            out=rap(yns, [[4, 16], [1, 4]]),
                            in0=rap(yu[:], [[4, 16], [1, 4]]),
                            in1=rap(rec[:], [[1, 16], [0, 4]]),
                            op=AL.mult)
                    pyn = ps_tr.tile([128, 128], bf, tag="pyn0")
                    nc.tensor.transpose(pyn[:], yn2[:], wsb['ident'][:])
                    nc.vector.tensor_copy(yrows[:, c0:c0 + 128], pyn[:])

                # ---- main pipeline per k-pair block ----
                pnum = ps_acc.tile([128, T], f32, tag="pnum")
                for b2 in range(NB):
                    w128 = slice(b2 * 128, (b2 + 1) * 128)
                    pr = ps_rv.tile([128, T], f32, tag="pr")
                    nc.tensor.matmul(pr[:], wsb['wk'][:], frt[:, b2, :],
                                     start=True, stop=False)
                    nc.tensor.matmul(pr[:], wsb['wq'][:, w128], xpt[:],
                                     start=False, stop=True)
                    pv = ps_rv.tile([128, T], f32, tag="pv")
                    if has_vb:
                        nc.tensor.matmul(pv[:], wsb['wv'][:], frt[:, b2, :],
                                         start=True, stop=False)
                        nc.tensor.matmul(pv[:], wsb['wvb'][:, w128], xpt[:],
                                         start=False, stop=True)
                    else:
                        nc.tensor.matmul(pv[:], wsb['wv'][:], frt[:, b2, :],
                                         start=True, stop=True)

                    ppe = ps_w.tile([128, T], f32, tag="ppe")
                    nc.tensor.matmul(ppe[:], wsb['wpe_pm'][:, w128], yrows[:],
                                     start=True, stop=True)
                    pem = blk.tile([128, T], bf, tag="pem")
                    nc.scalar.activation(out=pem[:], in_=ppe[:], func=AF.Relu)
                    ppb = ps_w.tile([128, T], f32, tag="ppe")
                    nc.tensor.matmul(ppb[:], wsb['wpe_pb'][:, w128], yrows[:],
                                     start=True, stop=True)
                    peb = blk.tile([128, T], bf, tag="peb")
                    nc.scalar.activation(out=peb[:], in_=ppb[:], func=AF.Relu)

                    dd = blk.tile([128, T], bf, tag="dd")
                    nc.vector.tensor_tensor(out=dd[:], in0=pr[:], in1=pem[:],
                                            op=AL.mult)
                    rr = blk.tile([128, T], bf, tag="rr")
                    nc.vector.tensor_tensor(out=rr[:], in0=dd[:], in1=peb[:],
                                            op=AL.add)
                    vv = blk.tile([128, T], bf, tag="vv")
                    nc.vector.tensor_tensor(out=vv[:], in0=pv[:], in1=peb[:],
                                            op=AL.add)

                    pw1 = ps_w.tile([128, T], f32, tag="pw1")
                    nc.tensor.matmul(pw1[:], wsb['we1'][:], rr[:],
                                     start=True, stop=True)
                    r1 = blk.tile([128, T], bf, tag="r1")
                    nc.scalar.activation(out=r1[:], in_=pw1[:], func=AF.Relu,
                                         bias=wsb['b1'][:], scale=1.0)
                    pw2 = ps_w.tile([128, T], f32, tag="pw1")
                    nc.tensor.matmul(pw2[:], wsb['we2'][:], r1[:],
                                     start=True, stop=True)
                    ew = blk.tile([128, T], bf, tag="ew")
                    nc.scalar.activation(out=ew[:], in_=pw2[:], func=AF.Exp,
                                         bias=wsb['b2'][:], scale=1.0)

                    nm = blk.tile([128, T], bf, tag="nm")
                    nc.vector.tensor_tensor(out=nm[:], in0=ew[:], in1=vv[:],
                                            op=AL.mult)
                    nc.tensor.matmul(pnum[0:64, :], wsb['wones'][:], nm[:],
                                     start=(b2 == 0), stop=(b2 == NB - 1),
                                     skip_group_check=True)
                    nc.tensor.matmul(pnum[64:128, :], wsb['wones'][:], ew[:],
                                     start=(b2 == 0), stop=(b2 == NB - 1),
                                     skip_group_check=True)

                # ---- feature = relu(num/den); final projection ----
                rden = mid.tile([64, T], bf, tag="rden")
                nc.vector.reciprocal(out=rden[:], in_=pnum[64:128, :])
                ff = mid.tile([FEATP, T], bf, tag="ff")
                nc.vector.scalar_tensor_tensor(
                    out=ff[0:64, :], in0=pnum[0:64, :], scalar=0.0,
                    in1=rden[:], op0=AL.max, op1=AL.mult)
                if has_reb:
                    nc.vector.memset(ff[64:65, :], 1.0)

                pout = ps_acc.tile([128, NSUB * 64], f32, tag="pout")
                for s in range(NSUB):
                    nc.tensor.matmul(pout[:, s * 64:(s + 1) * 64],
                                     ff[:, s * 128:(s + 1) * 128], wsb['wre'][:],
                                     start=True, stop=True)
                osb = mid.tile([128, NSUB * 64], f32, tag="osb")
                nc.vector.tensor_copy(osb[:], pout[:])
                nc.sync.dma_start(
                    out=bass.AP(tensor=out_d[:].tensor, offset=ms * 64,
                            ap=[[64, 128], [128 * 64, NSUB], [1, 64]]),
                    in_=rap(osb[:], [[64, NSUB], [1, 64]]))

    nc.finalize()
    return nc


LAST_RESULT = None


def kernel(**inputs):
    import os
    from concourse.bass_utils import run_bass_kernel_spmd

    W = _derived_weights(inputs)
    xp, fr = _host_tensors(inputs)
    Bl = xp.shape[0]
    Mloc = xp.shape[2]
    has_vb = 'wvb' in W
    has_reb = '_has_reb' in W
    wshapes = {k: v.shape for k, v in W.items() if not k.startswith('_')}

    key = (Mloc, has_vb, has_reb)
    if key not in _CACHE:
        _CACHE[key] = build_kernel(Mloc, has_vb, has_reb, wshapes)
    nc = _CACHE[key]

    in_maps = []
    for b in range(Bl):
        import ml_dtypes
        m = {"xp": xp[b], "fr": fr[b]}
        for name in wshapes:
            m["w_" + name] = W[name].astype(ml_dtypes.bfloat16)
        in_maps.append(m)

    res = run_bass_kernel_spmd(nc, in_maps, core_ids=list(range(Bl)),
                               trace=bool(os.environ.get("KERNEL_TRACE")))
    global LAST_RESULT
    LAST_RESULT = res
    out = np.stack([res.results[b]["out"] for b in range(Bl)], axis=0)
    return out.astype(np.float32)
